# revision 39
# baseline (speedup 1.0000x reference)
"""BitTransformerBlock on 8 Trainium2 NeuronCores.

Token-parallel sharding: the flattened (B*S)=4096 tokens are split 512 per
core; cores 0-3 hold batch 0, cores 4-7 batch 1.  Each core computes LN1 and
the q/k/v projections for its own tokens.  K^T and a ones-augmented V are
AllGathered (replica groups [0..3], [4..7]) as two separate collectives so K
arrives early; each core keeps its OWN K/V in SBUF and unpacks only the three
REMOTE peers' blocks via partition_id()-derived dynamic DMA offsets.  The
own-block part of attention (scores, exp, AV partials) runs while the
collectives are in flight, absorbing both the transfer time and any cross-core
launch skew.  Everything downstream (out-proj, LN2, quantized FFN) is
token-local.

All transposes (nx, hq, y1q) are SBUF->SBUF XBAR DMA transposes - no DRAM
round trips.  The softmax denominator rides as a ones-column appended to V
(row 64 of the AV matmul); its reciprocal is broadcast across partitions with
a K=1 ones-matmul on the PE instead of a DRAM bounce.

Precision: PE matmuls in bf16 with fp32 PSUM accumulation.  Scores run as
full K=128 contractions against half-zeroed q copies (qza/qzb) so the PE
array stays fully active and the HAM clock gate holds 2.4 GHz.  The BitNet
FFN uses exact int8 x ternary semantics (both exactly representable in bf16);
dequant scales applied per token afterwards.  Softmax without max subtraction
(logits are small).  rstd via Sqrt activation + DVE reciprocal.
"""

import numpy as np
import ml_dtypes

import concourse.bacc as bacc
import concourse.bass as bass
import concourse.mybir as mybir
import concourse.tile as tile
from concourse.bass_interp import get_hw_module
from concourse.bass_utils import run_bass_kernel_spmd

F32 = mybir.dt.float32
BF16 = mybir.dt.bfloat16
AF = mybir.ActivationFunctionType
OP = mybir.AluOpType

N_CORES = 8
B, S, D, H, FF = 2, 2048, 1024, 16, 4096
HD = D // H                 # 64
NTOK = B * S                # 4096
TOK = NTOK // N_CORES       # 512 tokens per core
TCH = TOK // 128            # 4 token chunks per core
DCH = D // 128              # 8
FFCH = FF // 128            # 32
NKC = S // 128              # 16 key chunks per batch
RKC = NKC - TCH             # 12 remote key chunks
GROUPS = [[0, 1, 2, 3], [4, 5, 6, 7]]
CPB = 4                     # cores per batch
VW = H * (HD + 1)           # 1040: V with a ones column per head
EPS = 1e-5
MAGIC = 12582912.0          # 1.5 * 2**23: fp32 round-to-nearest-even trick
INV_SQRT_HD = 1.0 / 8.0


def _bcast_part(ap, parts):
    """View a [1, F] (or [F]) AP as [parts, F] via a zero-stride partition dim."""
    inner = [list(e) for e in ap.ap if e[1] != 1] or [[1, 1]]
    return bass.AP(tensor=ap.tensor, offset=ap.offset, ap=[[0, parts]] + inner)


def build_program(s1, s2, biases, sim_gelu=False):
    nc = bacc.Bacc("TRN2", target_bir_lowering=False, debug=False,
                   num_devices=N_CORES)

    x_in = nc.dram_tensor("x_sh", [TOK, D], F32, kind="ExternalInput")
    wq_in = nc.dram_tensor("wqT", [D, D], BF16, kind="ExternalInput")
    wk_in = nc.dram_tensor("wkT", [D, D], BF16, kind="ExternalInput")
    wv_in = nc.dram_tensor("wvT", [D, D], BF16, kind="ExternalInput")
    wo_in = nc.dram_tensor("woT", [D, D], BF16, kind="ExternalInput")
    w1_in = nc.dram_tensor("w1T", [D, FF], BF16, kind="ExternalInput")
    w2_in = nc.dram_tensor("w2T", [FF, D], BF16, kind="ExternalInput")
    out_d = nc.dram_tensor("out", [TOK, D], F32, kind="ExternalOutput")

    ext = {}
    if biases["ln1_g"]:
        ext["ln1_g"] = nc.dram_tensor("ln1_g", [D], F32, kind="ExternalInput")
    if biases["ln1_b"]:
        ext["ln1_b"] = nc.dram_tensor("ln1_b", [D], F32, kind="ExternalInput")
    if biases["ln2_g"]:
        ext["ln2_g"] = nc.dram_tensor("ln2_g", [D], F32, kind="ExternalInput")
    if biases["ln2_b"]:
        ext["ln2_b"] = nc.dram_tensor("ln2_b", [D], F32, kind="ExternalInput")
    if biases["in_proj_b"]:
        ext["in_b"] = nc.dram_tensor("in_b", [3 * D], F32, kind="ExternalInput")
    if biases["out_proj_b"]:
        ext["out_b"] = nc.dram_tensor("out_b", [D], F32, kind="ExternalInput")
    if biases["b1"]:
        ext["b1"] = nc.dram_tensor("b1", [FF], F32, kind="ExternalInput")
    if biases["b2"]:
        ext["b2"] = nc.dram_tensor("b2", [D], F32, kind="ExternalInput")

    with tile.TileContext(nc) as tc:
        _emit(nc, tc, x_in, wq_in, wk_in, wv_in, wo_in, w1_in, w2_in, out_d,
              ext, s1, s2, biases, sim_gelu)
    nc.compile()
    return nc


def _emit(nc, tc, x_in, wq_in, wk_in, wv_in, wo_in, w1_in, w2_in, out_d,
          ext, s1, s2, biases, sim_gelu=False):
    gelu_func = AF.Tanh if sim_gelu else AF.Gelu
    from contextlib import ExitStack

    es_top = ExitStack()
    dram = es_top.enter_context(tc.tile_pool(name="dram", bufs=1, space="DRAM"))
    const = es_top.enter_context(tc.tile_pool(name="const", bufs=1))
    stats = es_top.enter_context(tc.tile_pool(name="stats", bufs=4))
    ptop = es_top.enter_context(tc.tile_pool(name="ptop", bufs=1))

    KBLK = D * TOK                  # K^T block, bf16 elements
    VBLK = TOK * VW                 # Vaug block
    k_bounce = dram.tile([KBLK], BF16)
    v_bounce = dram.tile([VBLK], BF16)
    k_all = dram.tile([CPB * KBLK], BF16)
    v_all = dram.tile([CPB * VBLK], BF16)

    x2 = ptop.tile([128, TCH, D], F32, tag="x2")

    eps_t = const.tile([128, 1], F32)
    nc.vector.memset(eps_t[:], EPS)
    magic_t = const.tile([128, 1], F32)
    nc.vector.memset(magic_t[:], MAGIC)
    ones64 = const.tile([1, 64], BF16)
    nc.vector.memset(ones64[:], 1.0)
    dq1 = const.tile([128, TCH], F32, tag="dq1")
    dq2 = const.tile([128, TCH], F32, tag="dq2")
    am8 = const.tile([128, TCH, 8], F32, tag="am8")

    def load_bcast(name, width, src_ap):
        t = const.tile([128, width], F32, tag=f"bc_{name}")
        nc.sync.dma_start(out=t[:], in_=_bcast_part(src_ap, 128))
        return t

    g1_bc = load_bcast("g1", D, ext["ln1_g"][:]) if biases["ln1_g"] else None
    b1ln_bc = load_bcast("b1ln", D, ext["ln1_b"][:]) if biases["ln1_b"] else None
    g2_bc = load_bcast("g2", D, ext["ln2_g"][:]) if biases["ln2_g"] else None
    b2ln_bc = load_bcast("b2ln", D, ext["ln2_b"][:]) if biases["ln2_b"] else None
    bv_bc = (load_bcast("bv", D, ext["in_b"][2 * D:3 * D])
             if biases["in_proj_b"] else None)
    bo_bc = load_bcast("bo", D, ext["out_b"][:]) if biases["out_proj_b"] else None
    bf1_bc = load_bcast("bf1", FF, ext["b1"][:]) if biases["b1"] else None
    bf2_bc = load_bcast("bf2", D, ext["b2"][:]) if biases["b2"] else None
    if biases["in_proj_b"]:
        bq_fm = const.tile([128, DCH], F32, tag="bq_fm")
        nc.sync.dma_start(out=bq_fm[:], in_=ext["in_b"][0:D].rearrange("(c p) -> p c", p=128))
        bk_fm = const.tile([128, DCH], F32, tag="bk_fm")
        nc.sync.dma_start(out=bk_fm[:], in_=ext["in_b"][D:2 * D].rearrange("(c p) -> p c", p=128))

    def layer_norm_chunk(src_ap, g_bc, b_bc, out_tile):
        st = stats.tile([128, 2, 6], F32, tag="bnst")
        nc.vector.bn_stats(out=st[:, 0, :], in_=src_ap[:, 0:512])
        nc.vector.bn_stats(out=st[:, 1, :], in_=src_ap[:, 512:1024])
        mv = stats.tile([128, 2], F32, tag="mv")
        nc.vector.bn_aggr(out=mv[:], in_=st[:])
        r = stats.tile([128, 1], F32, tag="rstd")
        nc.scalar.activation(out=r[:], in_=mv[:, 1:2], func=AF.Sqrt, bias=eps_t[:])
        nc.vector.reciprocal(out=r[:], in_=r[:])
        nc.vector.tensor_scalar(out=out_tile, in0=src_ap, scalar1=mv[:, 0:1],
                                scalar2=r[:], op0=OP.subtract, op1=OP.mult)
        if g_bc is not None:
            nc.vector.tensor_mul(out=out_tile, in0=out_tile, in1=g_bc[:])
        if b_bc is not None:
            nc.vector.tensor_add(out=out_tile, in0=out_tile, in1=b_bc[:])

    # ================= phase A: load x, LN1, projections =================
    es_A = ExitStack()
    pA = es_A.enter_context(tc.tile_pool(name="pA", bufs=1))
    x_sb = pA.tile([128, TCH, D], F32, tag="x")
    kT_own = pA.tile([128, DCH, TOK], BF16, tag="kTo")
    vaug_own = pA.tile([128, TCH, VW], BF16, tag="vao")
    # q feature-major, two half-zeroed copies: qza has head-b rows zeroed,
    # qzb head-a rows.  Scores then run as full K=128 contractions (keeps the
    # PE array fully active so the HAM clock gate stays at 2.4 GHz) with a
    # single shared lhsT per key chunk.
    qza = pA.tile([128, DCH, TOK], BF16, tag="qza")
    qzb = pA.tile([128, DCH, TOK], BF16, tag="qzb")
    oT = pA.tile([128, DCH, TOK], BF16, tag="oT")
    wo_sb = pA.tile([128, DCH, D], BF16, tag="wo")

    es_W = ExitStack()
    pW = es_W.enter_context(tc.tile_pool(name="pW", bufs=2))
    pNX = es_W.enter_context(tc.tile_pool(name="pNX", bufs=1))
    pNXc = es_W.enter_context(tc.tile_pool(name="pNXc", bufs=4))
    es_Wps = ExitStack()
    psP = es_Wps.enter_context(tc.tile_pool(name="psP", bufs=4, space="PSUM"))

    x_v = x_in.rearrange("(i p) d -> p i d", p=128)
    for i in range(TCH):
        nc.sync.dma_start(out=x_sb[:, i, :], in_=x_v[:, i, :])

    wk_sb = pW.tile([128, DCH, D], BF16, tag="w_in", name="wk")
    nc.sync.dma_start(out=wk_sb[:], in_=wk_in.rearrange("(c p) f -> p c f", p=128))

    nxT = pNX.tile([128, DCH, TOK], BF16, tag="nxT")
    _sid = nc.enter_named_scope("ln1", False)
    for i in range(TCH):
        nxc = pNXc.tile([128, D], BF16, tag="nx", name=f"nx{i}")
        layer_norm_chunk(x_sb[:, i, :], g1_bc, b1ln_bc, nxc[:])
        nc.sync.dma_start_transpose(out=nxT[:, :, i * 128:(i + 1) * 128],
                                    in_=nxc[:])
    # HAM warmup: ~4us of throwaway matmuls gated only on the first LN1
    # transpose, so kproj starts at 2.4 GHz instead of 1.2.
    for w in range(40):
        wps = psP.tile([128, 128], F32, tag="warm", name=f"warm{w}")
        nc.tensor.matmul(wps[:], lhsT=wk_sb[:, w % 4, 0:128],
                         rhs=nxT[:, w % 8, 0:128], start=True, stop=True)
    nc.leave_named_scope("ln1", _sid[0] if isinstance(_sid, tuple) else _sid, False)

    # ---- K projection (feature-major) -> kT_own + bounce + collective ----
    _sid = nc.enter_named_scope("kproj", False)
    for fo in range(DCH):
        ps = psP.tile([128, 512], F32, tag="psP")
        for dc in range(DCH):
            nc.tensor.matmul(ps[:], lhsT=wk_sb[:, dc, fo * 128:(fo + 1) * 128],
                             rhs=nxT[:, dc, :], start=(dc == 0), stop=(dc == DCH - 1))
        if biases["in_proj_b"]:
            nc.scalar.activation(out=kT_own[:, fo, :], in_=ps[:], func=AF.Identity,
                                 bias=bk_fm[:, fo:fo + 1])
        else:
            nc.vector.tensor_copy(out=kT_own[:, fo, :], in_=ps[:])
    nc.sync.dma_start(
        out=k_bounce[:].rearrange("(c p t) -> p c t", p=128, t=TOK),
        in_=kT_own[:])
    nc.leave_named_scope("kproj", _sid[0] if isinstance(_sid, tuple) else _sid, False)

    _sid = nc.enter_named_scope("collK", False)
    nc.gpsimd.collective_compute(
        "AllGather", OP.bypass, replica_groups=GROUPS,
        ins=[k_bounce.opt()], outs=[k_all.opt()])
    nc.leave_named_scope("collK", _sid[0] if isinstance(_sid, tuple) else _sid, False)

    # ---- V projection (token-major, ones-interleaved) + bounce + coll ----
    _sid = nc.enter_named_scope("vproj", False)
    wv_sb = pW.tile([128, DCH, D], BF16, tag="w_in", name="wv")
    nc.sync.dma_start(out=wv_sb[:], in_=wv_in.rearrange("(c p) f -> p c f", p=128))
    vao4 = vaug_own[:].rearrange("p i (h w) -> p i h w", w=HD + 1)
    nc.vector.memset(vao4[:, :, :, HD:HD + 1], 1.0)
    for to in range(TCH):
        for f2 in range(2):
            ps = psP.tile([128, 512], F32, tag="psP")
            for dc in range(DCH):
                nc.tensor.matmul(ps[:], lhsT=nxT[:, dc, to * 128:(to + 1) * 128],
                                 rhs=wv_sb[:, dc, f2 * 512:(f2 + 1) * 512],
                                 start=(dc == 0), stop=(dc == DCH - 1))
            dst = vao4[:, to, f2 * 8:(f2 + 1) * 8, 0:HD]
            src = ps[:].rearrange("p (h w) -> p h w", w=HD)
            if biases["in_proj_b"]:
                nc.vector.tensor_add(out=dst, in0=src,
                                     in1=bv_bc[:, f2 * 512:(f2 + 1) * 512]
                                     .rearrange("p (h w) -> p h w", w=HD))
            else:
                nc.vector.tensor_copy(out=dst, in_=src)
    nc.sync.dma_start(
        out=v_bounce[:].rearrange("(i p f) -> p i f", p=128, f=VW),
        in_=vaug_own[:])
    nc.leave_named_scope("vproj", _sid[0] if isinstance(_sid, tuple) else _sid, False)

    _sid = nc.enter_named_scope("collV", False)
    nc.gpsimd.collective_compute(
        "AllGather", OP.bypass, replica_groups=GROUPS,
        ins=[v_bounce.opt()], outs=[v_all.opt()])
    nc.leave_named_scope("collV", _sid[0] if isinstance(_sid, tuple) else _sid, False)

    # wo prefetch: DMA engines are idle while the collective runs
    wo_v = wo_in.rearrange("(c p) f -> p c f", p=128)
    for dc in range(DCH):
        nc.sync.dma_start(out=wo_sb[:, dc, :], in_=wo_v[:, dc, :])

    # ---- Q projection (feature-major) ----
    _sid = nc.enter_named_scope("qproj", False)
    wq_sb = pW.tile([128, DCH, D], BF16, tag="w_in", name="wq")
    nc.sync.dma_start(out=wq_sb[:], in_=wq_in.rearrange("(c p) f -> p c f", p=128))
    nc.vector.memset(qza[64:128, :, :], 0.0)
    nc.vector.memset(qzb[0:64, :, :], 0.0)
    for fo in range(DCH):
        ps = psP.tile([128, 512], F32, tag="psP")
        for dc in range(DCH):
            nc.tensor.matmul(ps[:], lhsT=wq_sb[:, dc, fo * 128:(fo + 1) * 128],
                             rhs=nxT[:, dc, :], start=(dc == 0), stop=(dc == DCH - 1))
        if biases["in_proj_b"]:
            nc.scalar.activation(out=qza[0:64, fo, :], in_=ps[0:64, :],
                                 func=AF.Identity, bias=bq_fm[0:64, fo:fo + 1])
            nc.scalar.activation(out=qzb[64:128, fo, :], in_=ps[64:128, :],
                                 func=AF.Identity, bias=bq_fm[64:128, fo:fo + 1])
        else:
            nc.vector.tensor_copy(out=qza[0:64, fo, :], in_=ps[0:64, :])
            nc.vector.tensor_copy(out=qzb[64:128, fo, :], in_=ps[64:128, :])
    nc.leave_named_scope("qproj", _sid[0] if isinstance(_sid, tuple) else _sid, False)
    es_Wps.close()
    es_W.close()

    # ================= phase B: attention =================
    es_B = ExitStack()
    pB = es_B.enter_context(tc.tile_pool(name="pB", bufs=1))
    pE = es_B.enter_context(tc.tile_pool(name="pE", bufs=16))
    pOA = es_B.enter_context(tc.tile_pool(name="pOA", bufs=2))
    ps_s = es_B.enter_context(tc.tile_pool(name="ps_s", bufs=2, space="PSUM"))
    ps_av = es_B.enter_context(tc.tile_pool(name="ps_av", bufs=3, space="PSUM"))
    ps_bc = es_B.enter_context(tc.tile_pool(name="ps_bc", bufs=1, space="PSUM"))

    KT_rem = pB.tile([128, DCH, RKC, 128], BF16, tag="KTr")
    Vaug_rem = pB.tile([128, RKC, VW], BF16, tag="Var")
    oacc = pB.tile([65, H, TOK], BF16, tag="oacc")
    dsb_all = pB.tile([1, H, TOK], BF16, tag="dsba")

    # unpack the three REMOTE peers' blocks (dynamic per-core offsets)
    _sid = nc.enter_named_scope("unpack", False)
    pid = nc.partition_id()
    for j in range(1, CPB):
        peer = (pid + j) % CPB
        nc.sync.dma_start(
            out=KT_rem[:, :, (j - 1) * TCH:j * TCH, :],
            in_=k_all[bass.DynSlice(peer * KBLK, KBLK)]
            .rearrange("(c p t) -> p c t", p=128, t=TOK))
    for j in range(1, CPB):
        peer = (pid + j) % CPB
        nc.sync.dma_start(
            out=Vaug_rem[:, (j - 1) * TCH:j * TCH, :],
            in_=v_all[bass.DynSlice(peer * VBLK, VBLK)]
            .rearrange("(i p f) -> p i f", p=128, f=VW))
    nc.leave_named_scope("unpack", _sid[0] if isinstance(_sid, tuple) else _sid, False)

    def score_pair(pss, lhs_src, j2, hp):
        """Score matmuls for head pair hp into pss[0/1][:, j2, :].  Full K=128
        contraction with a shared lhsT; the half-zeroed q copies select the
        head."""
        nc.tensor.matmul(pss[0][:, j2, :], lhsT=lhs_src,
                         rhs=qza[:, hp, :], start=True, stop=True)
        nc.tensor.matmul(pss[1][:, j2, :], lhsT=lhs_src,
                         rhs=qzb[:, hp, :], start=True, stop=True)

    # ---- own-block attention (no collective dependency) ----
    _sid = nc.enter_named_scope("attno", False)
    for hp in range(H // 2):
        pavs = [ps_av.tile([128, 512], F32, tag="pav", name=f"pavo{hp}_{jh}")
                for jh in range(2)]
        for g in range(2):
            pss = [ps_s.tile([128, 2, 512], F32, tag="pss", name=f"psso{hp}_{g}_{i}")
                   for i in range(2)]
            for j2 in range(2):
                tc_i = g * 2 + j2
                score_pair(pss, kT_own[:, hp, tc_i * 128:(tc_i + 1) * 128], j2, hp)
            es = []
            for jh in range(2):
                e = pE.tile([128, 2, 512], BF16, tag="e", name=f"eo{hp}_{g}_{jh}")
                nc.scalar.activation(out=e[:], in_=pss[jh][:], func=AF.Exp,
                                     scale=INV_SQRT_HD)
                es.append(e)
            for jh in range(2):
                h = 2 * hp + jh
                for j2 in range(2):
                    tc_i = g * 2 + j2
                    nc.tensor.matmul(
                        pavs[jh][0:65, :],
                        lhsT=vao4[:, tc_i, h, :],
                        rhs=es[jh][:, j2, :],
                        start=(g == 0 and j2 == 0), stop=(g == 1 and j2 == 1),
                        skip_group_check=True)
        for jh in range(2):
            nc.vector.tensor_copy(out=oacc[:, 2 * hp + jh, :], in_=pavs[jh][0:65, :])
    nc.leave_named_scope("attno", _sid[0] if isinstance(_sid, tuple) else _sid, False)

    # ---- remote-block attention + combine ----
    _sid = nc.enter_named_scope("attnr", False)
    var3 = Vaug_rem[:].rearrange("p k (h w) -> p k h w", w=HD + 1)
    for hp in range(H // 2):
        pavs = [ps_av.tile([128, 512], F32, tag="pav", name=f"pavr{hp}_{jh}")
                for jh in range(2)]
        for g in range(6):
            pss = [ps_s.tile([128, 2, 512], F32, tag="pss", name=f"pssr{hp}_{g}_{i}")
                   for i in range(2)]
            for j2 in range(2):
                rk = g * 2 + j2
                score_pair(pss, KT_rem[:, hp, rk, :], j2, hp)
            es = []
            for jh in range(2):
                e = pE.tile([128, 2, 512], BF16, tag="e", name=f"er{hp}_{g}_{jh}")
                nc.scalar.activation(out=e[:], in_=pss[jh][:], func=AF.Exp,
                                     scale=INV_SQRT_HD)
                es.append(e)
            for jh in range(2):
                h = 2 * hp + jh
                for j2 in range(2):
                    rk = g * 2 + j2
                    nc.tensor.matmul(
                        pavs[jh][0:65, :],
                        lhsT=var3[:, rk, h, :],
                        rhs=es[jh][:, j2, :],
                        start=(g == 0 and j2 == 0), stop=(g == 5 and j2 == 1),
                        skip_group_check=True)
        for jh in range(2):
            h = 2 * hp + jh
            # fp32 denominator + reciprocal inline on DVE (it has slack here);
            # bf16 combined numerators written back into oacc in place
            den = pOA.tile([1, TOK], F32, tag="den")
            nc.vector.tensor_add(out=den[:], in0=pavs[jh][64:65, :],
                                 in1=oacc[64:65, h, :])
            nc.vector.reciprocal(out=den[:], in_=den[:])
            nc.vector.tensor_copy(out=dsb_all[:, h, :], in_=den[:])
            nc.vector.tensor_add(out=oacc[:, h, :], in0=pavs[jh][0:65, :],
                                 in1=oacc[:, h, :])
    # deferred: per-head PE ones-broadcast of 1/den and the final multiply,
    # kept out of the attention PE stream so a slow reciprocal never blocks it
    for h in range(H):
        hp, jh = divmod(h, 2)
        bc = ps_bc.tile([64, 512], F32, tag="bc", name=f"bc{h}")
        nc.tensor.matmul(bc[:], lhsT=ones64[:], rhs=dsb_all[:, h, :],
                         start=True, stop=True)
        nc.vector.tensor_mul(out=oT[jh * 64:jh * 64 + 64, hp, :],
                             in0=oacc[0:64, h, :], in1=bc[:])
    nc.leave_named_scope("attnr", _sid[0] if isinstance(_sid, tuple) else _sid, False)

    es_B.close()

    # ================= phase C: out_proj + residual =================
    _sid = nc.enter_named_scope("outproj", False)
    es_6 = ExitStack()
    ps6 = es_6.enter_context(tc.tile_pool(name="ps6", bufs=4, space="PSUM"))
    for to in range(TCH):
        for f2 in range(2):
            ps = ps6.tile([128, 512], F32, tag="ps6")
            for dc in range(DCH):
                nc.tensor.matmul(ps[:], lhsT=oT[:, dc, to * 128:(to + 1) * 128],
                                 rhs=wo_sb[:, dc, f2 * 512:(f2 + 1) * 512],
                                 start=(dc == 0), stop=(dc == DCH - 1))
            dst = x2[:, to, f2 * 512:(f2 + 1) * 512]
            nc.vector.tensor_add(out=dst, in0=ps[:],
                                 in1=x_sb[:, to, f2 * 512:(f2 + 1) * 512])
            if biases["out_proj_b"]:
                nc.vector.tensor_add(out=dst, in0=dst,
                                     in1=bo_bc[:, f2 * 512:(f2 + 1) * 512])
    es_6.close()
    es_A.close()
    nc.leave_named_scope("outproj", _sid[0] if isinstance(_sid, tuple) else _sid, False)

    # ================= phase D: LN2 + quant + FFN =================
    es_F = ExitStack()
    pF = es_F.enter_context(tc.tile_pool(name="pF", bufs=1))
    pW1 = es_F.enter_context(tc.tile_pool(name="pW1", bufs=2))
    pYT = es_F.enter_context(tc.tile_pool(name="pYT", bufs=2))
    pQ = es_F.enter_context(tc.tile_pool(name="pQ", bufs=1))
    pLN = es_F.enter_context(tc.tile_pool(name="pLN", bufs=1))
    pQF = es_F.enter_context(tc.tile_pool(name="pQF", bufs=2))
    pOut = es_F.enter_context(tc.tile_pool(name="pOut", bufs=2))

    w2_sb = pF.tile([128, FFCH, D], BF16, tag="w2")
    w2_v = w2_in.rearrange("(c p) f -> p c f", p=128)
    hqT = pF.tile([128, DCH, TOK], BF16, tag="hqT")
    y1g = pF.tile([128, TCH, FF], BF16, tag="y1g")
    w1_v = w1_in.rearrange("(c p) f -> p c f", p=128)

    # LN2 + act_quant (token-major) + transpose to hqT
    _sid = nc.enter_named_scope("ln2q", False)
    for to in range(TCH):
        ht = pLN.tile([128, D], F32, tag="ht")
        layer_norm_chunk(x2[:, to, :], g2_bc, b2ln_bc, ht[:])
        am = stats.tile([128, 1], F32, tag="am")
        nc.vector.tensor_reduce(out=am[:], in_=ht[:], axis=mybir.AxisListType.X,
                                op=OP.max, apply_absolute_value=True)
        nc.vector.tensor_scalar_max(out=am[:], in0=am[:], scalar1=EPS)
        sc = stats.tile([128, 1], F32, tag="sc")
        nc.vector.reciprocal(out=sc[:], in_=am[:])
        nc.vector.tensor_scalar_mul(out=dq1[:, to:to + 1], in0=am[:],
                                    scalar1=float(s1) / 127.0)
        nc.vector.tensor_scalar_mul(out=sc[:], in0=sc[:], scalar1=127.0)
        rq = pLN.tile([128, D], F32, tag="rq")
        nc.vector.tensor_scalar(out=rq[:], in0=ht[:], scalar1=sc[:],
                                scalar2=magic_t[:], op0=OP.mult, op1=OP.add)
        hqt = pLN.tile([128, D], BF16, tag="hq")
        nc.vector.tensor_scalar_sub(out=hqt[:], in0=rq[:], scalar1=MAGIC)
        nc.sync.dma_start_transpose(out=hqT[:, :, to * 128:(to + 1) * 128],
                                    in_=hqt[:])
    nc.leave_named_scope("ln2q", _sid[0] if isinstance(_sid, tuple) else _sid, False)

    out_v = out_d.rearrange("(i p) d -> p i d", p=128)
    y1qTs = {}

    # ---- mm1: ffo-outer so w1 streams through SBUF in 1MB chunks ----
    _sid = nc.enter_named_scope("ffn1", False)
    es_p8 = ExitStack()
    ps8 = es_p8.enter_context(tc.tile_pool(name="ps8", bufs=4, space="PSUM"))
    for ffo in range(8):
        w1c = pW1.tile([128, DCH, 512], BF16, tag="w1c", name=f"w1c{ffo}")
        nc.sync.dma_start(out=w1c[:], in_=w1_v[:, :, ffo * 512:(ffo + 1) * 512])
        if 1 <= ffo <= 4:
            q = ffo - 1
            nc.sync.dma_start(out=w2_sb[:, q * 8:(q + 1) * 8, :],
                              in_=w2_v[:, q * 8:(q + 1) * 8, :])
        for to in range(TCH):
            ps = ps8.tile([128, 512], F32, tag="ps8")
            for dc in range(DCH):
                nc.tensor.matmul(ps[:], lhsT=hqT[:, dc, to * 128:(to + 1) * 128],
                                 rhs=w1c[:, dc, :],
                                 start=(dc == 0), stop=(dc == DCH - 1))
            dst = y1g[:, to, ffo * 512:(ffo + 1) * 512]
            if biases["b1"]:
                tmp = pQF.tile([128, 512], F32, tag="tmp1")
                nc.vector.scalar_tensor_tensor(
                    out=tmp[:], in0=ps[:], scalar=dq1[:, to:to + 1],
                    in1=bf1_bc[:, ffo * 512:(ffo + 1) * 512],
                    op0=OP.mult, op1=OP.add)
                nc.scalar.activation(out=dst, in_=tmp[:], func=gelu_func)
            else:
                nc.scalar.activation(out=dst, in_=ps[:], func=gelu_func,
                                     scale=dq1[:, to:to + 1])
            nc.vector.tensor_reduce(out=am8[:, to, ffo:ffo + 1], in_=dst,
                                    axis=mybir.AxisListType.X, op=OP.max,
                                    apply_absolute_value=True)
    es_p8.close()
    nc.leave_named_scope("ffn1", _sid[0] if isinstance(_sid, tuple) else _sid, False)

    _sid = nc.enter_named_scope("ffn2", False)
    es_p9 = ExitStack()
    ps9 = es_p9.enter_context(tc.tile_pool(name="ps9", bufs=4, space="PSUM"))

    def emit_quant(to):
        am = stats.tile([128, 1], F32, tag="am2")
        nc.vector.tensor_reduce(out=am[:], in_=am8[:, to, :],
                                axis=mybir.AxisListType.X, op=OP.max)
        nc.vector.tensor_scalar_max(out=am[:], in0=am[:], scalar1=EPS)
        sc = stats.tile([128, 1], F32, tag="sc2")
        nc.vector.reciprocal(out=sc[:], in_=am[:])
        nc.vector.tensor_scalar_mul(out=dq2[:, to:to + 1], in0=am[:],
                                    scalar1=float(s2) / 127.0)
        nc.vector.tensor_scalar_mul(out=sc[:], in0=sc[:], scalar1=127.0)
        y1q = pQ.tile([128, FF], BF16, tag="y1q", name=f"y1q{to}")
        for q in range(4):
            sl = slice(q * 1024, (q + 1) * 1024)
            rq = pQF.tile([128, 1024], F32, tag="rqf")
            nc.vector.tensor_scalar(out=rq[:], in0=y1g[:, to, sl], scalar1=sc[:],
                                    scalar2=magic_t[:], op0=OP.mult, op1=OP.add)
            nc.vector.tensor_scalar_sub(out=y1q[:, sl], in0=rq[:], scalar1=MAGIC)
        y1qT = pYT.tile([128, FFCH, 128], BF16, tag="y1qT", name=f"y1qT{to}")
        nc.sync.dma_start_transpose(out=y1qT[:], in_=y1q[:])
        y1qTs[to] = y1qT

    def emit_mm2(to):
        y1qT = y1qTs.pop(to)
        for f2 in range(2):
            ps = ps9.tile([128, 512], F32, tag="ps9")
            for fc in range(FFCH):
                nc.tensor.matmul(ps[:], lhsT=y1qT[:, fc, :],
                                 rhs=w2_sb[:, fc, f2 * 512:(f2 + 1) * 512],
                                 start=(fc == 0), stop=(fc == FFCH - 1))
            outt = pOut.tile([128, 512], F32, tag="outt")
            nc.vector.scalar_tensor_tensor(
                out=outt[:], in0=ps[:], scalar=dq2[:, to:to + 1],
                in1=x2[:, to, f2 * 512:(f2 + 1) * 512], op0=OP.mult, op1=OP.add)
            if biases["b2"]:
                nc.vector.tensor_add(out=outt[:], in0=outt[:],
                                     in1=bf2_bc[:, f2 * 512:(f2 + 1) * 512])
            nc.sync.dma_start(out=out_v[:, to, f2 * 512:(f2 + 1) * 512],
                              in_=outt[:])

    emit_quant(0)
    emit_quant(1)
    emit_mm2(0)
    emit_quant(2)
    emit_mm2(1)
    emit_quant(3)
    emit_mm2(2)
    emit_mm2(3)
    es_p9.close()
    nc.leave_named_scope("ffn2", _sid[0] if isinstance(_sid, tuple) else _sid, False)

    es_F.close()
    es_top.close()


_CACHE = {}


def _prepare(inputs):
    bf = ml_dtypes.bfloat16
    x = np.ascontiguousarray(np.asarray(inputs["x"], dtype=np.float32))
    in_w = np.asarray(inputs["in_proj_w"], dtype=np.float32)
    out_w = np.asarray(inputs["out_proj_w"], dtype=np.float32)
    w1 = np.asarray(inputs["w1"], dtype=np.float32)
    w2 = np.asarray(inputs["w2"], dtype=np.float32)

    s1 = float(max(np.mean(np.abs(w1), dtype=np.float32), EPS))
    s2 = float(max(np.mean(np.abs(w2), dtype=np.float32), EPS))
    t1 = np.clip(np.round(w1 / np.float32(s1)), -1.0, 1.0).astype(np.float32)
    t2 = np.clip(np.round(w2 / np.float32(s2)), -1.0, 1.0).astype(np.float32)

    host = {
        "wqT": np.ascontiguousarray(in_w[0:D].T).astype(bf),
        "wkT": np.ascontiguousarray(in_w[D:2 * D].T).astype(bf),
        "wvT": np.ascontiguousarray(in_w[2 * D:3 * D].T).astype(bf),
        "woT": np.ascontiguousarray(out_w.T).astype(bf),
        "w1T": np.ascontiguousarray(t1.T).astype(bf),
        "w2T": np.ascontiguousarray(t2.T).astype(bf),
    }

    def nz(a):
        return bool(np.any(np.asarray(a) != 0.0))

    biases = {
        "ln1_g": bool(np.any(np.asarray(inputs["ln1_g"]) != 1.0)),
        "ln1_b": nz(inputs["ln1_b"]),
        "ln2_g": bool(np.any(np.asarray(inputs["ln2_g"]) != 1.0)),
        "ln2_b": nz(inputs["ln2_b"]),
        "in_proj_b": nz(inputs["in_proj_b"]),
        "out_proj_b": nz(inputs["out_proj_b"]),
        "b1": nz(inputs["b1"]),
        "b2": nz(inputs["b2"]),
    }
    extra = {}
    if biases["ln1_g"]:
        extra["ln1_g"] = np.asarray(inputs["ln1_g"], np.float32)
    if biases["ln1_b"]:
        extra["ln1_b"] = np.asarray(inputs["ln1_b"], np.float32)
    if biases["ln2_g"]:
        extra["ln2_g"] = np.asarray(inputs["ln2_g"], np.float32)
    if biases["ln2_b"]:
        extra["ln2_b"] = np.asarray(inputs["ln2_b"], np.float32)
    if biases["in_proj_b"]:
        extra["in_b"] = np.asarray(inputs["in_proj_b"], np.float32)
    if biases["out_proj_b"]:
        extra["out_b"] = np.asarray(inputs["out_proj_b"], np.float32)
    if biases["b1"]:
        extra["b1"] = np.asarray(inputs["b1"], np.float32)
    if biases["b2"]:
        extra["b2"] = np.asarray(inputs["b2"], np.float32)

    x_flat = x.reshape(NTOK, D)
    in_maps = []
    for c in range(N_CORES):
        m = {"x_sh": np.ascontiguousarray(x_flat[c * TOK:(c + 1) * TOK])}
        m.update(host)
        m.update(extra)
        in_maps.append(m)
    return in_maps, s1, s2, biases


def get_program(s1, s2, biases, for_hw=True, sim_gelu=False):
    key = (round(s1, 12), round(s2, 12), tuple(sorted(biases.items())), for_hw,
           sim_gelu)
    if key not in _CACHE:
        nc = build_program(s1, s2, biases, sim_gelu=sim_gelu)
        if for_hw:
            nc.m = get_hw_module(nc.m)
        _CACHE[key] = nc
    return _CACHE[key]


def kernel(**inputs):
    in_maps, s1, s2, biases = _prepare(inputs)
    nc = get_program(s1, s2, biases, for_hw=True)
    res = run_bass_kernel_spmd(nc, in_maps, list(range(N_CORES)))
    out = np.concatenate([res.results[c]["out"] for c in range(N_CORES)], axis=0)
    return out.reshape(B, S, D).astype(np.float32)
